# revision 13
# baseline (speedup 1.0000x reference)
"""Trainium2 Bass kernel for nn_DA_conv1D (dynamic depthwise conv1d + 1x1 conv
+ channel-attention gate), data-parallel over batch on 8 NeuronCores.

Shapes (hardcoded): x0 [32, 64, 16384] f32, x1 [32, 64] f32.
Each core handles 4 samples, organized as 2 "pairs" of 2 samples so the
128 SBUF partitions hold (2 samples x 64 channels).

Per pair, per 2048-wide chunk (SBUF layout [128 part, L free]), split into
two 1024-wide PSUM tiles (2 banks each, ps1/ps2 double-buffered = 8 banks):
  ps1 = sum_j diag(kern_j) @ x_shift_j   (PE; tap-loop OUTER so the two
                                          N=512 matmuls per tap share one
                                          LDWEIGHTS and weight loads hide
                                          under the previous matmul)
  lr  = lrelu(ps1)                       (ACT Prelu over 1024, PSUM->SBUF,
                                          bf16 out)
  ps2 = blockdiag(conv_w) @ lr           (PE, 2x N=512 bf16 matmuls)
  out = x0 * att + ps2                   (DVE scalar_tensor_tensor over
                                          1024, bf16 out)

x0 is pre-cast to bf16 on host and DMAed once per chunk (used both for
the matmul path and the residual read). The output is stored as bf16 and
upcast to fp32 on host: halves both directions of HBM traffic.
The tiny dynamic-weight math (h = lrelu(x1 W1^T), kern = h W2^T, SE gate
att = sigmoid(lrelu(x1 ca_w1^T) ca_w2^T)) is computed on host in fp32 and
shipped as per-core diagonal/gate tensors (a few hundred KB).
"""

import os
import sys

for _p in ("/opt/trn_rl_repo", "/root/.axon_site/_ro/trn_rl_repo"):
    if os.path.isdir(_p) and _p not in sys.path:
        sys.path.append(_p)

import ml_dtypes
import numpy as np

import concourse.bacc as bacc
import concourse.tile as tile
from concourse import mybir
from concourse.bass_utils import run_bass_kernel_spmd

B, C, L, K = 32, 64, 16384, 3
N_CORES = 8
SAMPLES_PER_CORE = B // N_CORES          # 4
PAIRS = SAMPLES_PER_CORE // 2            # 2
P = 128                                  # SBUF partitions = 2 samples x 64 ch
CHUNK = 2048                             # max chunk (SBUF tile size)
# tapered schedule: small chunks at the edges shrink the pipeline
# fill/drain (first matmul waits only on a 0.13 MB DMA; final store is small)
PAIR_CHUNKS = [
    # pair 0: trickle start — tiny chunks arrive fast and warm the PE on
    # real work while the input stream builds its prefetch lead
    [128, 128, 256, 512, 1024] + [2048] * 7,
    [2048] * 7 + [1536, 512],           # pair 1: end-tapered
]
MTILE = 1024                             # ACT/DVE tile width (2 PSUM banks)
NTILE = 512                              # matmul moving width (PSUM bank)
N_CHUNKS = L // CHUNK
F32 = mybir.dt.float32
BF16 = mybir.dt.bfloat16
BF16_NP = ml_dtypes.bfloat16

TRACE = False          # test harness flips this to profile
USE_LRELU = True       # HW Prelu activation (CoreSim lacks it; see simcheck)
LAST_RESULT = None     # BassKernelResults of the most recent run

_COMPILED = {}         # (use_lrelu,) -> compiled Bacc program


def _build_program(use_lrelu):
    nc = bacc.Bacc("TRN2", target_bir_lowering=False, debug=False,
                   num_devices=N_CORES)

    x0b = nc.dram_tensor("x0b", [PAIRS, P, L], BF16,
                         kind="ExternalInput").ap()
    # diag kernels pre-flattened per partition: [(pair, tap) -> 128 cols]
    diags = nc.dram_tensor("diags", [P, PAIRS * K * P], BF16,
                           kind="ExternalInput").ap()
    # scal[:, 0:PAIRS] = att per pair; scal[:, PAIRS:2*PAIRS] = prelu bias
    # (-sum_j kern_j * d, the depthwise compensation for the host-side
    #  x0 + d shift that folds conv_b into the residual term)
    scal = nc.dram_tensor("scal", [P, 2 * PAIRS], F32,
                          kind="ExternalInput").ap()
    # bf16(d) per pair: halo value so the padded edge taps cancel exactly
    dcol = nc.dram_tensor("dcol", [PAIRS, P, 1], BF16,
                          kind="ExternalInput").ap()
    wblk = nc.dram_tensor("wblk", [P, P], BF16, kind="ExternalInput").ap()
    out = nc.dram_tensor("out", [PAIRS, P, L], BF16, kind="ExternalOutput").ap()

    mult = mybir.AluOpType.mult
    add = mybir.AluOpType.add
    Relu = mybir.ActivationFunctionType.Relu
    Prelu = mybir.ActivationFunctionType.Prelu
    Ident = mybir.ActivationFunctionType.Identity

    with tile.TileContext(nc) as tc:
        with (
            tc.tile_pool(name="consts", bufs=1) as consts,
            tc.tile_pool(name="xbf", bufs=10) as xbf_pool,
            tc.tile_pool(name="lr", bufs=4) as lr_pool,
            tc.tile_pool(name="r9", bufs=4) as r9_pool,
            tc.tile_pool(name="outc", bufs=6) as out_pool,
            tc.tile_pool(name="ps1", bufs=2, space="PSUM") as ps1_pool,
            tc.tile_pool(name="ps2", bufs=2, space="PSUM") as ps2_pool,
        ):
            # critical-path loads first: pair-0 diag weights, halo column,
            # first input chunk. Everything else (pair-1 diags, wblk, scal,
            # chunk prefetches) follows so the first matmul's dependencies
            # aren't starved behind the input-chunk descriptor flood.
            diag_t = consts.tile([P, PAIRS * K * P], BF16)
            nc.scalar.dma_start(diag_t[:, :K * P], diags[:, :K * P])
            dcol_t = consts.tile([P, PAIRS], BF16)
            for p in range(PAIRS):
                nc.scalar.dma_start(dcol_t[:, p:p + 1], dcol[p])

            sz0 = PAIR_CHUNKS[0][0]
            first_xbf = xbf_pool.tile([P, CHUNK + 4], BF16, tag="xbf")
            nc.sync.dma_start(first_xbf[:, 2:sz0 + 3],
                              x0b[0, :, 0:sz0 + 1])

            nc.scalar.dma_start(diag_t[:, K * P:], diags[:, K * P:])
            wblk_t = consts.tile([P, P], BF16)
            nc.scalar.dma_start(wblk_t[:], wblk[:])
            scal_t = consts.tile([P, 2 * PAIRS], F32)
            nc.scalar.dma_start(scal_t[:], scal[:])
            att = [scal_t[:, p:p + 1] for p in range(PAIRS)]
            pb = [scal_t[:, PAIRS + p:PAIRS + p + 1] for p in range(PAIRS)]

            # PE warm-up: dummy matmuls bridge the startup DMA window so the
            # tensor engine is at full clock (HAM) when real data arrives.
            dum = consts.tile([P, NTILE], BF16)
            nc.gpsimd.memset(dum[:], 0)
            ps_w = ps1_pool.tile([P, MTILE], F32, tag="ps")
            for w in range(8):
                nc.tensor.matmul(ps_w[:, (w % 2) * 256:(w % 2) * 256 + 256],
                                 dum[:, 0:P], dum[:, 0:256],
                                 start=True, stop=True)

            for p in range(PAIRS):
                lo = 0
                last_c = len(PAIR_CHUNKS[p]) - 1
                for c, csz in enumerate(PAIR_CHUNKS[p]):
                    # xbf[:, i] = x0[lo + i - 2]; i=0 never read
                    if p == 0 and c == 0:
                        xbf = first_xbf
                    else:
                        xbf = xbf_pool.tile([P, CHUNK + 4], BF16, tag="xbf")
                        # alternate the two HWDGE rings (sync/scalar) so
                        # dispatch serialization halves and input spreads
                        # over two logical DMA queues
                        eng = nc.sync if (c % 2 == 0) else nc.scalar
                        if c == 0:
                            eng.dma_start(xbf[:, 2:csz + 3],
                                          x0b[p, :, 0:csz + 1])
                        elif c == last_c:
                            eng.dma_start(xbf[:, 1:csz + 2],
                                          x0b[p, :, lo - 1:lo + csz])
                        else:
                            eng.dma_start(xbf[:, 1:csz + 3],
                                          x0b[p, :, lo - 1:lo + csz + 1])

                    outc = out_pool.tile([P, CHUNK], BF16, tag="outc")
                    for t in range((csz + MTILE - 1) // MTILE):
                        u = t * MTILE
                        msz = min(MTILE, csz - u)
                        nh = (msz + NTILE - 1) // NTILE
                        ps1 = ps1_pool.tile([P, MTILE], F32, tag="ps")
                        # tap loop OUTER: one LDWEIGHTS per tap, hidden
                        # under the previous tap's matmuls.
                        # Edge columns (x[-1], x[L]) read the halo value from
                        # dcol_t via N=1 matmuls: no cross-engine halo fill.
                        for j in range(K):
                            dslc = diag_t[:, (p * K + j) * P:
                                          (p * K + j + 1) * P]
                            st = (j == 0)
                            sp = (j == K - 1)
                            for h in range(nh):
                                w = min(NTILE, msz - h * NTILE)
                                c0 = u + h * NTILE        # chunk-local col
                                lo_cut = (j == 0 and c == 0 and c0 == 0)
                                hi_cut = (j == K - 1 and c == last_c
                                          and c0 + w == csz)
                                if lo_cut:
                                    nc.tensor.matmul(
                                        ps1[:, 0:1], dslc,
                                        dcol_t[:, p:p + 1],
                                        start=st, stop=sp)
                                    nc.tensor.matmul(
                                        ps1[:, h * NTILE + 1:h * NTILE + w],
                                        dslc, xbf[:, 2:1 + w],
                                        start=st, stop=sp)
                                elif hi_cut:
                                    nc.tensor.matmul(
                                        ps1[:, h * NTILE:h * NTILE + w - 1],
                                        dslc,
                                        xbf[:, c0 + 3:c0 + 2 + w],
                                        start=st, stop=sp)
                                    nc.tensor.matmul(
                                        ps1[:, h * NTILE + w - 1:
                                            h * NTILE + w],
                                        dslc, dcol_t[:, p:p + 1],
                                        start=st, stop=sp)
                                else:
                                    nc.tensor.matmul(
                                        ps1[:, h * NTILE:h * NTILE + w],
                                        dslc,
                                        xbf[:, c0 + 1 + j:c0 + 1 + j + w],
                                        start=st, stop=sp)
                        lr = lr_pool.tile([P, MTILE], BF16)
                        if use_lrelu:
                            nc.scalar.activation(lr[:, :msz], ps1[:, :msz],
                                                 Prelu, bias=pb[p], alpha=0.1)
                        else:
                            tt = r9_pool.tile([P, MTILE], F32, tag="tt")
                            nc.scalar.activation(tt[:, :msz], ps1[:, :msz],
                                                 Ident, bias=pb[p])
                            r9 = r9_pool.tile([P, MTILE], F32)
                            nc.scalar.activation(r9[:, :msz], tt[:, :msz],
                                                 Relu, scale=0.9)
                            nc.vector.scalar_tensor_tensor(
                                lr[:, :msz], tt[:, :msz], 0.1, r9[:, :msz],
                                op0=mult, op1=add)
                        ps2 = ps2_pool.tile([P, MTILE], F32)
                        for h in range(nh):
                            w = min(NTILE, msz - h * NTILE)
                            nc.tensor.matmul(
                                ps2[:, h * NTILE:h * NTILE + w],
                                wblk_t[:],
                                lr[:, h * NTILE:h * NTILE + w],
                                start=True, stop=True)
                        oslice = outc[:, u:u + msz]
                        nc.vector.scalar_tensor_tensor(
                            oslice, xbf[:, u + 2:u + 2 + msz],
                            att[p], ps2[:, :msz], op0=mult, op1=add)
                    seng = nc.gpsimd if (c % 2 == 0) else nc.sync
                    seng.dma_start(out[p, :, lo:lo + csz],
                                   outc[:, :csz])
                    lo += csz

    nc.compile()
    return nc


def _lrelu(x):
    return np.where(x >= 0, x, np.float32(0.1) * x)


def kernel(x0, x1, W1, W2, conv_w, conv_b, ca_w1, ca_w2):
    global LAST_RESULT
    x0 = np.ascontiguousarray(np.asarray(x0, dtype=np.float32))
    x1 = np.asarray(x1, dtype=np.float32)
    W1 = np.asarray(W1, dtype=np.float32)
    W2 = np.asarray(W2, dtype=np.float32)
    conv_w = np.asarray(conv_w, dtype=np.float32)
    conv_b = np.asarray(conv_b, dtype=np.float32)
    ca_w1 = np.asarray(ca_w1, dtype=np.float32)
    ca_w2 = np.asarray(ca_w2, dtype=np.float32)

    # dynamic depthwise kernels + SE gate (tiny, fp32 host math)
    h = _lrelu(x1 @ W1.T)                                   # [B, 64]
    kern = (h @ W2.T).reshape(B, C, K)                      # [B, C, K]
    att = 1.0 / (1.0 + np.exp(-(_lrelu(x1 @ ca_w1.T) @ ca_w2.T)))
    att = att.astype(np.float32)                            # [B, C]

    # block-diagonal 1x1-conv weight as lhsT: lhsT[k, m] = W[m, k]
    wblk_np = np.zeros((P, P), np.float32)
    wblk_np[:C, :C] = conv_w.T
    wblk_np[C:, C:] = conv_w.T
    wblk_np = wblk_np.astype(BF16_NP)

    key = (USE_LRELU,)
    if key not in _COMPILED:
        _COMPILED[key] = _build_program(USE_LRELU)
    nc = _COMPILED[key]

    biasP = np.tile(conv_b, 2).astype(np.float32)            # [P]
    in_maps = []
    for core in range(N_CORES):
        s0 = core * SAMPLES_PER_CORE
        diags_np = np.zeros((P, PAIRS * K * P), np.float32)
        scal_np = np.empty((P, 2 * PAIRS), np.float32)
        dcol_np = np.empty((PAIRS, P, 1), np.float32)
        dvals = np.empty((PAIRS, P), np.float32)
        for p in range(PAIRS):
            ka = kern[s0 + 2 * p]          # [C, K]
            kb = kern[s0 + 2 * p + 1]
            kern_bf = np.empty((P, K), np.float32)
            for j in range(K):
                s = (p * K + j) * P
                d = np.concatenate([ka[:, j], kb[:, j]])
                np.fill_diagonal(diags_np[:, s:s + P], d)
                kern_bf[:, j] = d.astype(BF16_NP).astype(np.float32)
            attp = np.concatenate([att[s0 + 2 * p], att[s0 + 2 * p + 1]])
            dp = biasP / attp                                 # [P]
            dvals[p] = dp
            dcol_np[p, :, 0] = dp
            scal_np[:, p] = attp
            # depthwise compensation: -sum_j bf16(kern_j) * d
            scal_np[:, PAIRS + p] = -(kern_bf.sum(axis=1) * dp)
        x0c = x0[s0:s0 + SAMPLES_PER_CORE].reshape(PAIRS, P, L)
        x0c = (x0c + dvals[:, :, None]).astype(BF16_NP)
        in_maps.append({
            "x0b": x0c,
            "diags": diags_np.astype(BF16_NP),
            "scal": scal_np,
            "dcol": dcol_np.astype(BF16_NP),
            "wblk": wblk_np,
        })

    res = run_bass_kernel_spmd(nc, in_maps, list(range(N_CORES)), trace=TRACE)
    LAST_RESULT = res

    full = np.empty((B, C, L), np.float32)
    for core in range(N_CORES):
        s0 = core * SAMPLES_PER_CORE
        full[s0:s0 + SAMPLES_PER_CORE] = (
            res.results[core]["out"].astype(np.float32)
            .reshape(SAMPLES_PER_CORE, C, L))
    return full
